# revision 7
# baseline (speedup 1.0000x reference)
"""GwcVolume (group-wise correlation volume) Bass kernel for Trainium2.

Problem: left/right features [2, 320, 96, 312] fp32, GROUP=40, cpg=8,
max_disp=48.  Output cost volume [2, 40, 48, 96, 312]:
    cost[b,g,d,h,w] = mean_c( l[b,g,c,h,w] * r[b,g,c,h,w-d] ),  0 for w<d.

Strategy (8 NeuronCores):
  - Shard the 80 (b,g) pairs across cores, 10 per core.  Each pair is fully
    independent (no collectives).
  - TensorE does all multiply-accumulate work as block-diagonal matmuls:
    for each (bg, h-group of 16), SBUF holds l as [128 = 16h x 8c, W] and a
    block-diagonal stationary image rs [128, 10*128] where the
    (unit, w'-block blk, h-quad q) stationary is
        rs[32q + 8hi + c, 128 blk + 32 hi + ww] = r[h, c, 32 blk + ww] / 0,
    h = 16 hg + 4 q + hi.  matmul (K=32 rows at strip 32q, M=128, N=79):
        out[(hi,ww), n] = sum_c r[h,c,w'0+ww] * l[h,c,w'0+n]
                        = cost[d=n-ww, h, w=w'0+n]  for 0 <= n-ww < 48.
    The 4 quads run on distinct PE row-strips and distinct PSUM banks,
    so they execute concurrently on the 32x32 sub-array grid.
  - v2: the block-diagonal image is built ON DEVICE: DRAM holds only the
    compact r [128, GU, 320] (4x fewer HBM bytes); per group, 4 DMAs (one
    per hi) write the diagonal blocks into persistent ping-pong rs tiles
    whose off-diagonal zeros are memset once at kernel start.
  - v2: PSUM is evacuated to a slot-major SBUF buffer by vector+scalar+
    gpsimd (3-way slot split).  The out-DMA extracts the valid d-band with
    51-wide windows per ww-quartet (partitions [32hi+4t, +4) share window
    [4t, 4t+51)), cutting the band overhead from 64/48 to 51/48.
  - The host does the final (free) rearrangement: band extraction
    (d = m - delta), zero triangle for w < d, and the layout transpose.
"""

import os

import numpy as np

# --- geometry (hardcoded for this problem) ---
B, G, CPG, H, W = 2, 40, 8, 96, 312
D = 48                      # max_disp
N_CORES = 8
PAIRS = B * G               # 80 (b,g) pairs
BG_PER_CORE = PAIRS // N_CORES  # 10
HGROUPS = H // 16           # 6 groups of 16 h's
NBLK = 10                   # w'-blocks of 32 (covers w' in [0, 320))
MBLK = 32                   # w' per block
WL = 368                    # padded l width in SBUF (max needed w = 366)
WR = 320                    # padded r width (312 + 8)
UNITS = BG_PER_CORE * HGROUPS   # 60 (bg, hgroup) units per core
RSW = NBLK * 128            # 1280 stationary-image cols per unit
GU = 6                      # units per DMA group (batch DMAs for efficiency)
NGROUPS = UNITS // GU       # 10
NWM = 80                    # matmul moving cols (max window 4*7+51 = 79)
WIN = 51                    # out-DMA window width (4 ww's + 47)
GH = GU // 2                # out-DMA half-group granularity

_NC_CACHE = {}


def _build_nc(dt_in_name="bfloat16", dt_out_name="bfloat16"):
    from concourse import bacc, mybir, tile
    import concourse.bass as bass  # noqa: F401

    dt_in = getattr(mybir.dt, dt_in_name)
    dt_out = getattr(mybir.dt, dt_out_name)
    f32 = mybir.dt.float32

    nc = bacc.Bacc("TRN2", target_bir_lowering=False, debug=False)
    l_dram = nc.dram_tensor(
        "l", [NGROUPS, 128, GU, W], dt_in, kind="ExternalInput")
    r_dram = nc.dram_tensor(
        "rc", [NGROUPS, 128, GU, WR], dt_in, kind="ExternalInput")
    # out: per (hi, t) the 4 partitions 32hi+4t+delta share slot window
    # [4t, 4t+51).  o[g, hi, t, delta, u, m, q, s] =
    #   cost[d = m-delta, h = 16 hg + 4 q + hi, w = 32 s + 4 t + delta]
    o_dram = nc.dram_tensor(
        "o", [NGROUPS, 4, 8, 4, GU, WIN, 4, NBLK], dt_out,
        kind="ExternalOutput")

    HB = NBLK // 2              # 5 blks per psum tile (one bank per quad)
    SEG = 96                    # col stride of a blk inside its bank
    # 2-way evac slot split (vector / scalar; gpsimd cannot read PSUM),
    # tuned for the engines' measured copy rates (~1.14 / 1.27 ns/el).
    SPLITS = (0, 44, NWM)

    with tile.TileContext(nc) as tc:
        with (
            tc.tile_pool(name="pp", bufs=1) as pp,
            tc.tile_pool(name="evp", bufs=3) as evp,
            tc.tile_pool(name="psp", bufs=2, space="PSUM") as psp,
        ):
            # persistent ping-pong tiles: l pad cols + rs off-diagonal
            # zeros are written once and survive all groups.
            lts, rts = [], []
            for pi in range(2):
                lt = pp.tile([128, GU, WL], dt_in, tag=f"l{pi}",
                             name=f"lt{pi}")
                rt = pp.tile([128, GU, RSW], dt_in, tag=f"r{pi}",
                             name=f"rt{pi}")
                nc.vector.memset(lt[:, :, W:], 0.0)
                # split the big rs memset across engines (one-time cost)
                nc.vector.memset(rt[:, :, 0:RSW // 2], 0.0)
                nc.gpsimd.memset(rt[:, :, RSW // 2:RSW], 0.0)
                lts.append(lt)
                rts.append(rt)

            for g in range(NGROUPS):
                lt, rt = lts[g % 2], rts[g % 2]
                # split loads in halves: the first half-group's compute can
                # start while the second half streams in
                for u0 in (0, GH):
                    nc.sync.dma_start(
                        lt[:, u0:u0 + GH, :W], l_dram[g, :, u0:u0 + GH, :])
                # block-diagonal build: per (q, hi) write the 8-partition
                # run rt[(q, hp=hi, c), u, (blk, hc=hi, ww)] = rc[...].
                # Contiguous partitions only -- partition-crossing DMA dims
                # break the tile dep tracker.
                for q in range(4):
                    for hi in range(4):
                        p0 = 32 * q + 8 * hi
                        dst = rt[p0:p0 + 8].rearrange(
                            "c u (blk hc ww) -> c u blk hc ww",
                            blk=NBLK, hc=4)[:, :, :, hi, :]
                        src = r_dram[g, p0:p0 + 8].rearrange(
                            "c u (blk ww) -> c u blk ww", blk=NBLK)
                        eng = nc.sync if (q + hi) % 2 else nc.scalar
                        eng.dma_start(dst, src)
                ev = evp.tile([128, GU, NWM, 4, NBLK], dt_out)
                for ui in range(GU):
                    for h in range(2):
                        # 5 blks share one psum tile: quad q in bank q,
                        # blk s at col 96*s
                        ps = psp.tile([128, 4, 512], f32)
                        for s in range(HB):
                            blk = h * HB + s
                            for q in range(4):
                                nc.tensor.matmul(
                                    out=ps[:, q, SEG * s:SEG * s + NWM],
                                    lhsT=rt[32 * q:32 * q + 32, ui,
                                            128 * blk:128 * blk + 128],
                                    rhs=lt[32 * q:32 * q + 32, ui,
                                           MBLK * blk:MBLK * blk + NWM],
                                    start=True,
                                    stop=True,
                                    tile_position=(32 * q, 0),
                                )
                        # [p, q, s, n] -> [p, n, q, s] slot-major store,
                        # 3-way slot split across vector/scalar/gpsimd
                        src = ps[:, :, 0:HB * SEG].rearrange(
                            "p q (s n) -> p n q s", n=SEG)
                        dst = ev[:, ui, :, :, h * HB:(h + 1) * HB]
                        for ei, eng_copy in enumerate(
                                (nc.vector.tensor_copy, nc.scalar.copy)):
                            s0, s1 = SPLITS[ei], SPLITS[ei + 1]
                            eng_copy(out=dst[:, s0:s1],
                                     in_=src[:, s0:s1])
                    # out-DMA per (half-group, hi, ww-quartet): partitions
                    # [32hi+4t, +4) read their 51-slot window [4t, 4t+51).
                    # hi in {0,1} are even-parity SDMA partitions, {2,3}
                    # odd: give each HWDGE ring (ACT / sync) one of each.
                    if ui == GH - 1 or ui == GU - 1:
                        u0 = ui + 1 - GH
                        for hi in range(4):
                            eng = nc.scalar if hi in (0, 3) else nc.sync
                            for t in range(8):
                                p0 = 32 * hi + 4 * t
                                eng.dma_start(
                                    o_dram[g, hi, t, :, u0:u0 + GH],
                                    ev[p0:p0 + 4, u0:u0 + GH,
                                       4 * t:4 * t + WIN, :, :])
    nc.compile()
    return nc


def _get_nc(key=("bfloat16", "bfloat16")):
    if key not in _NC_CACHE:
        _NC_CACHE[key] = _build_nc(*key)
    return _NC_CACHE[key]


def _pack_inputs(left, right, dt_np):
    """-> per-core in_maps; l pre-scaled by 1/cpg, r compact (padded to
    320 cols), both [NGROUPS, 128 = 16h x 8c, GU, W*]."""
    # [B, C, H, W] -> [B, G, cpg, H, W] -> [pair, H, cpg, W]
    l5 = left.reshape(B, G, CPG, H, W).transpose(0, 1, 3, 2, 4).reshape(
        PAIRS, H, CPG, W)
    r5 = right.reshape(B, G, CPG, H, W).transpose(0, 1, 3, 2, 4).reshape(
        PAIRS, H, CPG, W)
    lp = (l5 * (1.0 / CPG)).astype(dt_np)
    # l: [pair, H=6*16, cpg, W] -> per core [NGROUPS, 128, GU, W]
    lp = lp.reshape(N_CORES, NGROUPS, GU, 128, W).transpose(0, 1, 3, 2, 4)

    rp = np.zeros((PAIRS, H, CPG, WR), dtype=dt_np)
    rp[..., :W] = r5
    rp = rp.reshape(N_CORES, NGROUPS, GU, 128, WR).transpose(0, 1, 3, 2, 4)
    return [
        {"l": np.ascontiguousarray(lp[k]), "rc": np.ascontiguousarray(rp[k])}
        for k in range(N_CORES)
    ]


def _unpack_outputs(outs):
    """outs: 8 arrays [NGROUPS, 4, 8, 4, GU, WIN, 4, NBLK] -> [B,G,D,H,W].

    o[g, hi, t, delta, u, m, q, s] = cost[d = m - delta,
        h = 16*hg + 4q + hi, w = 32 s + 4 t + delta]  (unit = g*GU+u)
    valid for m - delta in [0, 48)."""
    O = np.stack([np.asarray(o, dtype=np.float32) for o in outs])
    # [8, NG, 4hi, 8t, 4delta, GU, WIN, 4q, NBLK]
    # -> unit-major [pair, hg, hi, t, delta, m, q, s]
    O = O.transpose(0, 1, 5, 2, 3, 4, 6, 7, 8).reshape(
        PAIRS, HGROUPS, 4, 8, 4, WIN, 4, NBLK)
    WPAD = 368  # max written w = 32*9 + 4*7 + 3 + 47 = 366
    final = np.zeros((PAIRS, D, H, WPAD), dtype=np.float32)
    s0, sd, sh, sw = (np.array(final.strides) // final.itemsize)
    st = np.lib.stride_tricks.as_strided
    it = final.itemsize
    a = np.array(O.strides) // it
    for q in range(4):
        for hi in range(4):
            for dl in range(4):
                h0 = 4 * q + hi
                # A[pair, hg, t, m] = O[pair, hg, hi, t, dl, m, q, :]
                # V[pair, hg, t, s, d] = A[..., t, dl + d] (d = m - dl)
                V = st(O[:, :, hi, :, dl, dl:, q, :],
                       shape=(PAIRS, HGROUPS, 8, NBLK, D),
                       strides=tuple(np.array(
                           [a[0], a[1], a[3], a[7], a[5]]) * it))
                # dest: final[pair, d, 16*hg + h0, 32*s + 4*t + dl + d]
                Dv = st(final[:, :, h0:, dl:],
                        shape=(PAIRS, HGROUPS, 8, NBLK, D),
                        strides=tuple(np.array(
                            [s0, 16 * sh, 4 * sw, MBLK * sw,
                             sd + sw]) * it))
                Dv[...] = V
    return final[:, :, :, :W].reshape(B, G, D, H, W)


def _install_profile_hook():
    """Make trace=True work when the image's antenv lacks axon_hooks."""
    import sys
    import types
    try:
        from antenv.axon_hooks import get_axon_ntff_profile_hook  # noqa: F401
        return
    except ImportError:
        pass
    if "/root/.axon_site" not in sys.path:
        sys.path.insert(0, "/root/.axon_site")
    from trn_agent_boot.trn_boot import _ntff_profile_via_ctypes
    hook = _ntff_profile_via_ctypes("/opt/axon/libaxon_pjrt.so")
    import antenv
    mod = types.ModuleType("antenv.axon_hooks")
    state = {"hook": hook}
    mod.get_axon_ntff_profile_hook = lambda: state["hook"]
    mod.set_axon_ntff_profile_hook = lambda h: state.update(hook=h)
    sys.modules["antenv.axon_hooks"] = mod
    antenv.axon_hooks = mod


def kernel(left_feature, right_feature, max_disp):
    import sys
    if "/opt/trn_rl_repo" not in sys.path:
        sys.path.insert(0, "/opt/trn_rl_repo")
    from concourse import bass_utils
    from concourse.bass_utils import run_bass_kernel_spmd

    left = np.asarray(left_feature, dtype=np.float32)
    right = np.asarray(right_feature, dtype=np.float32)
    assert int(max_disp) == D
    assert left.shape == (B, G * CPG, H, W)

    dt_in_name = os.environ.get("GWC_DT_IN", "bfloat16")
    dt_out_name = os.environ.get("GWC_DT_OUT", "bfloat16")
    if dt_in_name == "bfloat16":
        import ml_dtypes
        dt_np = ml_dtypes.bfloat16
    else:
        dt_np = np.float32
    nc = _get_nc((dt_in_name, dt_out_name))
    in_maps = _pack_inputs(left, right, dt_np)

    trace = bool(os.environ.get("GWC_PROFILE"))
    if trace:
        _install_profile_hook()
        bass_utils.upload_artifacts = lambda tmpdir: str(tmpdir)  # no bucket
    res = run_bass_kernel_spmd(
        nc, in_maps, core_ids=list(range(N_CORES)), trace=trace
    )
    if trace:
        kernel._last_profile = res
        print(f"[kernel] exec_time_ns={res.exec_time_ns} "
              f"mean={res.mean_exec_time_ns}", flush=True)
    outs = [res.results[k]["o"] for k in range(N_CORES)]
    return _unpack_outputs(outs)


# revision 13
# speedup vs baseline: 1.0656x; 1.0656x over previous
"""GwcVolume (group-wise correlation volume) Bass kernel for Trainium2.

Problem: left/right features [2, 320, 96, 312] fp32, GROUP=40, cpg=8,
max_disp=48.  Output cost volume [2, 40, 48, 96, 312]:
    cost[b,g,d,h,w] = mean_c( l[b,g,c,h,w] * r[b,g,c,h,w-d] ),  0 for w<d.

Strategy (8 NeuronCores):
  - Shard the 80 (b,g) pairs across cores, 10 per core.  Each pair is fully
    independent (no collectives).
  - TensorE does all multiply-accumulate work as block-diagonal matmuls:
    for each (bg, h-group of 16), SBUF holds l as [128 = 16h x 8c, W] and a
    block-diagonal stationary image rs [128, 10*128] where the
    (unit, w'-block blk, h-quad q) stationary is
        rs[32q + 8hi + c, 128 blk + 32 hi + ww] = r[h, c, 32 blk + ww] / 0,
    h = 16 hg + 4 q + hi.  matmul (K=32 rows at strip 32q, M=128, N=79):
        out[(hi,ww), n] = sum_c r[h,c,w'0+ww] * l[h,c,w'0+n]
                        = cost[d=n-ww, h, w=w'0+n]  for 0 <= n-ww < 48.
    The 4 quads run on distinct PE row-strips and distinct PSUM banks,
    so they execute concurrently on the 32x32 sub-array grid.
  - HBM holds only the COMPACT r [128, GU, 320] (4x fewer bytes than the
    block-diagonal image).  The otherwise-idle GPSIMD engine expands it
    on device:  rs[p, u, blk, hc, ww] = rc[p, u, blk, ww] * mask[p, hc],
    one broadcast-multiply per (group, unit).
  - PSUM is evacuated to a slot-major SBUF buffer by vector+scalar (slot
    ranges split 44/36 to balance their measured copy rates; gpsimd
    cannot read PSUM).  The out-DMA extracts the valid d-band with
    51-wide windows per ww-quartet (partitions [32hi+4t, +4) share slot
    window [4t, 4t+51)), cutting band overhead from 64/48 to 51/48 with
    fully contiguous 4-KB packets.
  - The host does the final (free) rearrangement: band extraction
    (d = m - delta), zero triangle for w < d, and the layout transpose.
"""

import os

import numpy as np

# --- geometry (hardcoded for this problem) ---
B, G, CPG, H, W = 2, 40, 8, 96, 312
D = 48                      # max_disp
N_CORES = 8
PAIRS = B * G               # 80 (b,g) pairs
BG_PER_CORE = PAIRS // N_CORES  # 10
HGROUPS = H // 16           # 6 groups of 16 h's
NBLK = 10                   # w'-blocks of 32 (covers w' in [0, 320))
MBLK = 32                   # w' per block
WL = 368                    # padded l width (312 + 56; max needed w = 366)
WR = 320                    # padded r width (312 + 8)
UNITS = BG_PER_CORE * HGROUPS   # 60 (bg, hgroup) units per core
RSW = NBLK * 128            # 1280 stationary-image cols per unit
GU = 6                      # units per DMA group (batch DMAs for efficiency)
NGROUPS = UNITS // GU       # 10
NWM = 80                    # matmul moving cols (max window 4*7+51 = 79)
WIN = 51                    # out-DMA window width (4 ww's + 47)
GH = GU // 2                # out-DMA half-group granularity

_NC_CACHE = {}


def _build_nc(dt_in_name="bfloat16", dt_out_name="bfloat16"):
    from concourse import bacc, mybir, tile
    import concourse.bass as bass  # noqa: F401

    dt_in = getattr(mybir.dt, dt_in_name)
    dt_out = getattr(mybir.dt, dt_out_name)
    f32 = mybir.dt.float32

    nc = bacc.Bacc("TRN2", target_bir_lowering=False, debug=False)
    l_dram = nc.dram_tensor(
        "l", [NGROUPS, 128, GU, WL], dt_in, kind="ExternalInput")
    r_dram = nc.dram_tensor(
        "rc", [NGROUPS, 128, GU, WR], dt_in, kind="ExternalInput")
    m_dram = nc.dram_tensor("mk", [128, 4], dt_in, kind="ExternalInput")
    # out: per (hi, t) the 4 partitions 32hi+4t+delta share slot window
    # [4t, 4t+51).  o[g, hi, t, delta, u, m, q, s] =
    #   cost[d = m-delta, h = 16 hg + 4 q + hi, w = 32 s + 4 t + delta + d]
    o_dram = nc.dram_tensor(
        "o", [NGROUPS, 4, 8, 4, GU, WIN, 4, NBLK], dt_out,
        kind="ExternalOutput")

    HB = NBLK // 2              # 5 blks per psum tile (one bank per quad)
    SEG = 96                    # col stride of a blk inside its bank
    # 2-way evac slot split (vector / scalar; gpsimd cannot read PSUM),
    # tuned for the engines' measured copy rates (~1.14 / 1.27 ns/el).
    SPLITS = (0, 44, NWM)

    with tile.TileContext(nc) as tc:
        with (
            tc.tile_pool(name="mp", bufs=1) as mp,
            tc.tile_pool(name="lp", bufs=2) as lp,
            tc.tile_pool(name="cp", bufs=2) as cp,
            tc.tile_pool(name="rp", bufs=2) as rp,
            tc.tile_pool(name="evp", bufs=3) as evp,
            tc.tile_pool(name="psp", bufs=2, space="PSUM") as psp,
        ):
            mask = mp.tile([128, 4], dt_in)
            nc.sync.dma_start(mask[:], m_dram[:])
            for g in range(NGROUPS):
                lt = lp.tile([128, GU, WL], dt_in)
                ct = cp.tile([128, GU, WR], dt_in)
                rt = rp.tile([128, GU, RSW], dt_in)
                # split loads in halves: the first half-group's build +
                # compute can start while the second half streams in
                for u0 in (0, GH):
                    nc.sync.dma_start(
                        lt[:, u0:u0 + GH, :], l_dram[g, :, u0:u0 + GH, :])
                    nc.scalar.dma_start(
                        ct[:, u0:u0 + GH, :], r_dram[g, :, u0:u0 + GH, :])
                # gpsimd expands compact r to the block-diagonal image,
                # one broadcast-multiply per unit
                for u in range(GU):
                    src = ct[:, u].rearrange(
                        "p (blk ww) -> p blk ww", blk=NBLK)
                    src_b = src.unsqueeze(2).broadcast_to(
                        [128, NBLK, 4, MBLK])
                    msk_b = mask.unsqueeze(1).unsqueeze(3).broadcast_to(
                        [128, NBLK, 4, MBLK])
                    dst = rt[:, u].rearrange(
                        "p (blk hc ww) -> p blk hc ww", blk=NBLK, hc=4)
                    nc.gpsimd.tensor_tensor(
                        out=dst, in0=src_b, in1=msk_b,
                        op=mybir.AluOpType.mult)
                ev = evp.tile([128, GU, NWM, 4, NBLK], dt_out)
                for ui in range(GU):
                    for h in range(2):
                        # 5 blks share one psum tile: quad q in bank q,
                        # blk s at col 96*s
                        ps = psp.tile([128, 4, 512], f32)
                        for s in range(HB):
                            blk = h * HB + s
                            for q in range(4):
                                nc.tensor.matmul(
                                    out=ps[:, q, SEG * s:SEG * s + NWM],
                                    lhsT=rt[32 * q:32 * q + 32, ui,
                                            128 * blk:128 * blk + 128],
                                    rhs=lt[32 * q:32 * q + 32, ui,
                                           MBLK * blk:MBLK * blk + NWM],
                                    start=True,
                                    stop=True,
                                    tile_position=(32 * q, 0),
                                )
                        # [p, q, s, n] -> [p, n, q, s] slot-major store,
                        # 2-way slot split across vector/scalar
                        src = ps[:, :, 0:HB * SEG].rearrange(
                            "p q (s n) -> p n q s", n=SEG)
                        dst = ev[:, ui, :, :, h * HB:(h + 1) * HB]
                        for ei, eng_copy in enumerate(
                                (nc.vector.tensor_copy, nc.scalar.copy)):
                            s0, s1 = SPLITS[ei], SPLITS[ei + 1]
                            eng_copy(out=dst[:, s0:s1],
                                     in_=src[:, s0:s1])
                    # out-DMA per (half-group, hi, ww-quartet): partitions
                    # [32hi+4t, +4) read their 51-slot window [4t, 4t+51).
                    # hi in {0,1} are even-parity SDMA partitions, {2,3}
                    # odd: give each HWDGE ring (ACT / sync) one of each.
                    if ui == GH - 1 or ui == GU - 1:
                        u0 = ui + 1 - GH
                        for hi in range(4):
                            eng = nc.scalar if hi in (0, 3) else nc.sync
                            for t in range(8):
                                p0 = 32 * hi + 4 * t
                                eng.dma_start(
                                    o_dram[g, hi, t, :, u0:u0 + GH],
                                    ev[p0:p0 + 4, u0:u0 + GH,
                                       4 * t:4 * t + WIN, :, :])
    nc.compile()
    return nc


def _get_nc(key=("bfloat16", "bfloat16")):
    if key not in _NC_CACHE:
        _NC_CACHE[key] = _build_nc(*key)
    return _NC_CACHE[key]


def _pack_inputs(left, right, dt_np):
    """-> per-core in_maps; l pre-scaled by 1/cpg (padded to 368 cols),
    r compact (padded to 320), both [NGROUPS, 128 = 16h x 8c, GU, W*],
    plus the block-diagonal selection mask [128, 4]."""
    # [B, C, H, W] -> [B, G, cpg, H, W] -> [pair, H, cpg, W]
    l5 = left.reshape(B, G, CPG, H, W).transpose(0, 1, 3, 2, 4).reshape(
        PAIRS, H, CPG, W)
    r5 = right.reshape(B, G, CPG, H, W).transpose(0, 1, 3, 2, 4).reshape(
        PAIRS, H, CPG, W)
    lp = np.zeros((PAIRS, H, CPG, WL), dtype=np.float32)
    lp[..., :W] = l5 * (1.0 / CPG)
    lp = lp.astype(dt_np)
    # l: [pair, H=6*16, cpg, WL] -> per core [NGROUPS, 128, GU, WL]
    lp = lp.reshape(N_CORES, NGROUPS, GU, 128, WL).transpose(0, 1, 3, 2, 4)

    rp = np.zeros((PAIRS, H, CPG, WR), dtype=dt_np)
    rp[..., :W] = r5
    rp = rp.reshape(N_CORES, NGROUPS, GU, 128, WR).transpose(0, 1, 3, 2, 4)

    mk = np.zeros((128, 4), dtype=dt_np)
    for q in range(4):
        for hi in range(4):
            mk[32 * q + 8 * hi:32 * q + 8 * hi + 8, hi] = 1
    return [
        {"l": np.ascontiguousarray(lp[k]),
         "rc": np.ascontiguousarray(rp[k]),
         "mk": mk}
        for k in range(N_CORES)
    ]


def _unpack_outputs(outs):
    """outs: 8 arrays [NGROUPS, 4, 8, 4, GU, WIN, 4, NBLK] -> [B,G,D,H,W].

    o[g, hi, t, delta, u, m, q, s] = cost[d = m - delta,
        h = 16*hg + 4q + hi, w = 32 s + 4 t + delta + d]  (unit = g*GU+u)
    valid for m - delta in [0, 48)."""
    O = np.stack([np.asarray(o, dtype=np.float32) for o in outs])
    # [8, NG, 4hi, 8t, 4delta, GU, WIN, 4q, NBLK]
    # -> unit-major [pair, hg, hi, t, delta, m, q, s]
    O = O.transpose(0, 1, 5, 2, 3, 4, 6, 7, 8).reshape(
        PAIRS, HGROUPS, 4, 8, 4, WIN, 4, NBLK)
    WPAD = 368  # max written w = 32*9 + 4*7 + 3 + 47 = 366
    final = np.zeros((PAIRS, D, H, WPAD), dtype=np.float32)
    s0, sd, sh, sw = (np.array(final.strides) // final.itemsize)
    st = np.lib.stride_tricks.as_strided
    it = final.itemsize
    a = np.array(O.strides) // it
    for q in range(4):
        for hi in range(4):
            for dl in range(4):
                h0 = 4 * q + hi
                # A[pair, hg, t, m] = O[pair, hg, hi, t, dl, m, q, :]
                # V[pair, hg, t, s, d] = A[..., t, dl + d] (d = m - dl)
                V = st(O[:, :, hi, :, dl, dl:, q, :],
                       shape=(PAIRS, HGROUPS, 8, NBLK, D),
                       strides=tuple(np.array(
                           [a[0], a[1], a[3], a[7], a[5]]) * it))
                # dest: final[pair, d, 16*hg + h0, 32*s + 4*t + dl + d]
                Dv = st(final[:, :, h0:, dl:],
                        shape=(PAIRS, HGROUPS, 8, NBLK, D),
                        strides=tuple(np.array(
                            [s0, 16 * sh, 4 * sw, MBLK * sw,
                             sd + sw]) * it))
                Dv[...] = V
    return final[:, :, :, :W].reshape(B, G, D, H, W)


def _install_profile_hook():
    """Make trace=True work when the image's antenv lacks axon_hooks."""
    import sys
    import types
    try:
        from antenv.axon_hooks import get_axon_ntff_profile_hook  # noqa: F401
        return
    except ImportError:
        pass
    if "/root/.axon_site" not in sys.path:
        sys.path.insert(0, "/root/.axon_site")
    from trn_agent_boot.trn_boot import _ntff_profile_via_ctypes
    hook = _ntff_profile_via_ctypes("/opt/axon/libaxon_pjrt.so")
    import antenv
    mod = types.ModuleType("antenv.axon_hooks")
    state = {"hook": hook}
    mod.get_axon_ntff_profile_hook = lambda: state["hook"]
    mod.set_axon_ntff_profile_hook = lambda h: state.update(hook=h)
    sys.modules["antenv.axon_hooks"] = mod
    antenv.axon_hooks = mod


def kernel(left_feature, right_feature, max_disp):
    import sys
    if "/opt/trn_rl_repo" not in sys.path:
        sys.path.insert(0, "/opt/trn_rl_repo")
    from concourse import bass_utils
    from concourse.bass_utils import run_bass_kernel_spmd

    left = np.asarray(left_feature, dtype=np.float32)
    right = np.asarray(right_feature, dtype=np.float32)
    assert int(max_disp) == D
    assert left.shape == (B, G * CPG, H, W)

    dt_in_name = os.environ.get("GWC_DT_IN", "bfloat16")
    dt_out_name = os.environ.get("GWC_DT_OUT", "bfloat16")
    if dt_in_name == "bfloat16":
        import ml_dtypes
        dt_np = ml_dtypes.bfloat16
    else:
        dt_np = np.float32
    nc = _get_nc((dt_in_name, dt_out_name))
    in_maps = _pack_inputs(left, right, dt_np)

    trace = bool(os.environ.get("GWC_PROFILE"))
    if trace:
        _install_profile_hook()
        bass_utils.upload_artifacts = lambda tmpdir: str(tmpdir)  # no bucket
    res = run_bass_kernel_spmd(
        nc, in_maps, core_ids=list(range(N_CORES)), trace=trace
    )
    if trace:
        kernel._last_profile = res
        print(f"[kernel] exec_time_ns={res.exec_time_ns} "
              f"mean={res.mean_exec_time_ns}", flush=True)
    outs = [res.results[k]["o"] for k in range(N_CORES)]
    return _unpack_outputs(outs)


# revision 19
# speedup vs baseline: 2.8626x; 2.6863x over previous
"""GwcVolume (group-wise correlation volume) Bass kernel for Trainium2.

Problem: left/right features [2, 320, 96, 312] fp32, GROUP=40, cpg=8,
max_disp=48.  Output cost volume [2, 40, 48, 96, 312]:
    cost[b,g,d,h,w] = mean_c( l[b,g,c,h,w] * r[b,g,c,h,w-d] ),  0 for w<d.

Strategy (8 NeuronCores):
  - Shard the 80 (b,g) pairs across cores, 10 per core.  Each pair is fully
    independent (no collectives).
  - TensorE does all multiply-accumulate work as block-diagonal matmuls:
    for each (bg, h-group of 16), SBUF holds l as [128 = 16h x 8c, W] and a
    block-diagonal stationary image rs [128, 10*128] where the
    (unit, w'-block blk, h-quad q) stationary is
        rs[32q + 8hi + c, 128 blk + 32 hi + ww] = r[h, c, 32 blk + ww] / 0,
    h = 16 hg + 4 q + hi.  matmul (K=32 rows at strip 32q, M=128, N=79):
        out[(hi,ww), n] = sum_c r[h,c,w'0+ww] * l[h,c,w'0+n]
                        = cost[d=n-ww, h, w=w'0+n]  for 0 <= n-ww < 48.
    The 4 quads run on distinct PE row-strips and distinct PSUM banks,
    so they execute concurrently on the 32x32 sub-array grid.
  - HBM holds only the COMPACT r [128, GU, 320] (4x fewer bytes than the
    block-diagonal image).  The otherwise-idle GPSIMD engine expands it on
    device as a uint32 bitwise-AND against a per-partition hc-selection
    mask (2 bf16 values per op), one instruction per (group, unit).
  - Band compaction via slot-major ev layout: stationary M-cols are
    permuted (in the on-device build) so PSUM partition p = 64k + 16hi + i
    holds ww = 16k + i, making the valid 48-wide band of partitions [0,64)
    sit inside moving cols [0,64) and of [64,128) inside [16,80).
    ev stores the matmul col index ("slot") OUTER-most per partition,
    so each half's 64-slot window is one contiguous run and the
    out-DMA extracts a 64/48 = 1.33x band with ~5KB packets.  (Narrower
    windows would need more dma_starts; at ~1us of engine issue time per
    dma_start, instruction count dominates any byte savings.)
  - PSUM is evacuated by vector+scalar with slot ranges split 42/38 to
    balance their measured copy rates (gpsimd cannot read PSUM).
  - The host does the final (free) rearrangement: band extraction
    (d = n - ww), zero triangle for w < d, and the layout transpose.
"""

import os

import numpy as np

# --- geometry (hardcoded for this problem) ---
B, G, CPG, H, W = 2, 40, 8, 96, 312
D = 48                      # max_disp
N_CORES = 8
PAIRS = B * G               # 80 (b,g) pairs
BG_PER_CORE = PAIRS // N_CORES  # 10
HGROUPS = H // 16           # 6 groups of 16 h's
NBLK = 10                   # w'-blocks of 32 (covers w' in [0, 320))
MBLK = 32                   # w' per block
NW = MBLK + D - 1           # 79 moving columns per matmul
WL = 368                    # padded l width (312 + 56; max needed w = 366)
WR = 320                    # padded r width (312 + 8)
UNITS = BG_PER_CORE * HGROUPS   # 60 (bg, hgroup) units per core
RSW = NBLK * 128            # 1280 stationary-image cols per unit
GU = 6                      # units per DMA group (batch DMAs for efficiency)
NGROUPS = UNITS // GU       # 10

_NC_CACHE = {}


def _build_nc(dt_in_name="bfloat16", dt_out_name="bfloat16"):
    from concourse import bacc, mybir, tile
    import concourse.bass as bass  # noqa: F401

    dt_in = getattr(mybir.dt, dt_in_name)
    dt_out = getattr(mybir.dt, dt_out_name)
    f32 = mybir.dt.float32
    u32 = mybir.dt.uint32

    nc = bacc.Bacc("TRN2", target_bir_lowering=False, debug=False)
    l_dram = nc.dram_tensor(
        "l", [NGROUPS, 128, GU, WL], dt_in, kind="ExternalInput")
    r_dram = nc.dram_tensor(
        "rc", [NGROUPS, 128, GU, WR], dt_in, kind="ExternalInput")
    m_dram = nc.dram_tensor("mk", [128, 4], dt_in, kind="ExternalInput")
    NWM = 80                    # matmul cols (covers window [16,80))
    o_dram = nc.dram_tensor(
        "o", [NGROUPS, 2, 64, GU, 64, 4, NBLK], dt_out,
        kind="ExternalOutput")

    HB = NBLK // 2              # 5 blks per psum tile (one bank per quad)
    SEG = 96                    # col stride of a blk inside its bank
    GH = GU // 2                # out-DMA half-group granularity
    # 2-way evac slot split (vector / scalar; gpsimd cannot read PSUM),
    # tuned for the engines' measured copy rates (~1.25 / 1.41 ns/el).
    SPLITS = (0, 42, NWM)

    with tile.TileContext(nc) as tc:
        with (
            tc.tile_pool(name="mp", bufs=1) as mp,
            tc.tile_pool(name="lp", bufs=2) as lp,
            tc.tile_pool(name="cp", bufs=2) as cp,
            tc.tile_pool(name="rp", bufs=2) as rp,
            tc.tile_pool(name="evp", bufs=3) as evp,
            tc.tile_pool(name="psp", bufs=2, space="PSUM") as psp,
        ):
            mask = mp.tile([128, 4], dt_in)
            nc.sync.dma_start(mask[:], m_dram[:])
            for g in range(NGROUPS):
                lt = lp.tile([128, GU, WL], dt_in)
                ct = cp.tile([128, GU, WR], dt_in)
                rt = rp.tile([128, GU, RSW], dt_in)
                # split loads in halves: the first half-group's build +
                # compute can start while the second half streams in
                for u0 in (0, GH):
                    nc.sync.dma_start(
                        lt[:, u0:u0 + GH, :], l_dram[g, :, u0:u0 + GH, :])
                    nc.scalar.dma_start(
                        ct[:, u0:u0 + GH, :], r_dram[g, :, u0:u0 + GH, :])
                # gpsimd expands compact r to the (slot-major permuted)
                # block-diagonal image, one broadcast-multiply per unit:
                #   rs[p, u, blk, k, hc, i] = rc[p, u, blk, k, i]
                #                             * mask[p, hc]
                # (Pool cannot do uint32 bitwise ops, so bf16 multiply.)
                for u in range(GU):
                    # (blk, k) merged into one dim: engine APs allow at
                    # most 3 free dims (TENSOR3D)
                    src = ct[:, u].rearrange(
                        "p (bk i) -> p bk i", bk=2 * NBLK)
                    src_b = src.unsqueeze(2).broadcast_to(
                        [128, 2 * NBLK, 4, 16])
                    msk_b = mask.unsqueeze(1).unsqueeze(3).broadcast_to(
                        [128, 2 * NBLK, 4, 16])
                    dst = rt[:, u].rearrange(
                        "p (bk hc i) -> p bk hc i", bk=2 * NBLK, hc=4)
                    nc.gpsimd.tensor_tensor(
                        out=dst, in0=src_b, in1=msk_b,
                        op=mybir.AluOpType.mult)
                ev = evp.tile([128, GU, NWM, 4, NBLK], dt_out)
                for ui in range(GU):
                    for h in range(2):
                        # 5 blks share one psum tile: quad q in bank q,
                        # blk s at col 96*s
                        ps = psp.tile([128, 4, 512], f32)
                        for s in range(HB):
                            blk = h * HB + s
                            for q in range(4):
                                nc.tensor.matmul(
                                    out=ps[:, q, SEG * s:SEG * s + NWM],
                                    lhsT=rt[32 * q:32 * q + 32, ui,
                                            128 * blk:128 * blk + 128],
                                    rhs=lt[32 * q:32 * q + 32, ui,
                                           MBLK * blk:MBLK * blk + NWM],
                                    start=True,
                                    stop=True,
                                    tile_position=(32 * q, 0),
                                )
                        # [p, q, s, n] -> [p, n, q, s] slot-major store,
                        # 2-way slot split across vector/scalar
                        src = ps[:, :, 0:HB * SEG].rearrange(
                            "p q (s n) -> p n q s", n=SEG)
                        dst = ev[:, ui, :, :, h * HB:(h + 1) * HB]
                        for ei, eng_copy in enumerate(
                                (nc.vector.tensor_copy, nc.scalar.copy)):
                            s0, s1 = SPLITS[ei], SPLITS[ei + 1]
                            eng_copy(out=dst[:, s0:s1],
                                     in_=src[:, s0:s1])
                    # out-DMA per (half-group, partition-half); each reads
                    # its half's contiguous 64-slot band window.  k=0
                    # targets even SDMA engines (p<64) and k=1 odd
                    # (p>=64): issue them on SEPARATE HWDGE rings
                    # (ACT / sync) so both engine-parity sets drain
                    # concurrently from independent descriptor streams
                    if ui == GH - 1 or ui == GU - 1:
                        u0 = ui + 1 - GH
                        for k in range(2):
                            eng = nc.scalar if k == 0 else nc.sync
                            eng.dma_start(
                                o_dram[g, k, :, u0:u0 + GH],
                                ev[64 * k:64 * k + 64, u0:u0 + GH,
                                   16 * k:16 * k + 64, :, :])
    nc.compile()
    return nc


def _get_nc(key=("bfloat16", "bfloat16")):
    if key not in _NC_CACHE:
        _NC_CACHE[key] = _build_nc(*key)
    return _NC_CACHE[key]


def _pack_inputs(left, right, dt_np):
    """-> per-core in_maps; l pre-scaled by 1/cpg (padded to 368 cols),
    r compact (padded to 320), both [NGROUPS, 128 = 16h x 8c, GU, W*],
    plus the uint32 block-diagonal selection mask [128, 4]."""
    # [B, C, H, W] -> [B, G, cpg, H, W] -> [pair, H, cpg, W]
    l5 = left.reshape(B, G, CPG, H, W).transpose(0, 1, 3, 2, 4).reshape(
        PAIRS, H, CPG, W)
    r5 = right.reshape(B, G, CPG, H, W).transpose(0, 1, 3, 2, 4).reshape(
        PAIRS, H, CPG, W)
    lp = np.zeros((PAIRS, H, CPG, WL), dtype=np.float32)
    lp[..., :W] = l5 * (1.0 / CPG)
    lp = lp.astype(dt_np)
    # l: [pair, H=6*16, cpg, WL] -> per core [NGROUPS, 128, GU, WL]
    lp = lp.reshape(N_CORES, NGROUPS, GU, 128, WL).transpose(0, 1, 3, 2, 4)

    rp = np.zeros((PAIRS, H, CPG, WR), dtype=dt_np)
    rp[..., :W] = r5
    rp = rp.reshape(N_CORES, NGROUPS, GU, 128, WR).transpose(0, 1, 3, 2, 4)

    mk = np.zeros((128, 4), dtype=dt_np)
    for q in range(4):
        for hi in range(4):
            mk[32 * q + 8 * hi:32 * q + 8 * hi + 8, hi] = 1
    return [
        {"l": np.ascontiguousarray(lp[k]),
         "rc": np.ascontiguousarray(rp[k]),
         "mk": mk}
        for k in range(N_CORES)
    ]


def _unpack_outputs(outs):
    """outs: 8 arrays [NGROUPS, 2, 64, GU, 64, 4, NBLK] -> [B,G,D,H,W]."""
    # dims: [g, k, (4hi,16i), ui, 64j, 4q, 10blk]; partition held
    # ww = 16k + i, stored slot j maps to matmul col n = 16k + j, so
    # d = n - ww = j - i (valid j in [i, i+48))
    O = np.stack([
        np.asarray(o, dtype=np.float32).transpose(0, 3, 1, 2, 4, 5, 6)
        for o in outs])
    # [8core, NG, GU, 2k, 4hi, 16i, 64j, 4q, 10blk] -> unit-major
    O = O.reshape(PAIRS, HGROUPS, 2, 4, 16, 64, 4, NBLK)
    WPAD = 368
    final = np.zeros((PAIRS, D, H, WPAD), dtype=np.float32)
    s0, sd, sh, sw = (np.array(final.strides) // final.itemsize)
    st = np.lib.stride_tricks.as_strided
    it = final.itemsize
    for q in range(4):
        for hi in range(4):
            for k in range(2):
                h0 = 4 * q + hi
                A = O[:, :, k, hi, :, :, q, :]  # [80, 6, 16i, 64j, 10blk]
                a = np.array(A.strides) // it
                # V[pair, hg, i, blk, d] = A[pair, hg, i, i+d, blk]
                V = st(A, shape=(PAIRS, HGROUPS, 16, NBLK, D),
                       strides=tuple(np.array([a[0], a[1], a[2] + a[3],
                                               a[4], a[3]]) * it))
                # dest: final[pair, d, 16*hg + h0, 32*blk + 16*k + i + d]
                Dv = st(final[:, :, h0:, 16 * k:],
                        shape=(PAIRS, HGROUPS, 16, NBLK, D),
                        strides=tuple(np.array([s0, 16 * sh, sw, MBLK * sw,
                                                sd + sw]) * it))
                Dv[...] = V
    return final[:, :, :, :W].reshape(B, G, D, H, W)


def _install_profile_hook():
    """Make trace=True work when the image's antenv lacks axon_hooks."""
    import sys
    import types
    try:
        from antenv.axon_hooks import get_axon_ntff_profile_hook  # noqa: F401
        return
    except ImportError:
        pass
    if "/root/.axon_site" not in sys.path:
        sys.path.insert(0, "/root/.axon_site")
    from trn_agent_boot.trn_boot import _ntff_profile_via_ctypes
    hook = _ntff_profile_via_ctypes("/opt/axon/libaxon_pjrt.so")
    import antenv
    mod = types.ModuleType("antenv.axon_hooks")
    state = {"hook": hook}
    mod.get_axon_ntff_profile_hook = lambda: state["hook"]
    mod.set_axon_ntff_profile_hook = lambda h: state.update(hook=h)
    sys.modules["antenv.axon_hooks"] = mod
    antenv.axon_hooks = mod


def kernel(left_feature, right_feature, max_disp):
    import sys
    if "/opt/trn_rl_repo" not in sys.path:
        sys.path.insert(0, "/opt/trn_rl_repo")
    from concourse import bass_utils
    from concourse.bass_utils import run_bass_kernel_spmd

    left = np.asarray(left_feature, dtype=np.float32)
    right = np.asarray(right_feature, dtype=np.float32)
    assert int(max_disp) == D
    assert left.shape == (B, G * CPG, H, W)

    dt_in_name = os.environ.get("GWC_DT_IN", "bfloat16")
    dt_out_name = os.environ.get("GWC_DT_OUT", "bfloat16")
    if dt_in_name == "bfloat16":
        import ml_dtypes
        dt_np = ml_dtypes.bfloat16
    else:
        dt_np = np.float32
    nc = _get_nc((dt_in_name, dt_out_name))
    in_maps = _pack_inputs(left, right, dt_np)

    trace = bool(os.environ.get("GWC_PROFILE"))
    if trace:
        _install_profile_hook()
        bass_utils.upload_artifacts = lambda tmpdir: str(tmpdir)  # no bucket
    res = run_bass_kernel_spmd(
        nc, in_maps, core_ids=list(range(N_CORES)), trace=trace
    )
    if trace:
        kernel._last_profile = res
        print(f"[kernel] exec_time_ns={res.exec_time_ns} "
              f"mean={res.mean_exec_time_ns}", flush=True)
    outs = [res.results[k]["o"] for k in range(N_CORES)]
    return _unpack_outputs(outs)


# revision 21
# speedup vs baseline: 2.9490x; 1.0302x over previous
"""GwcVolume (group-wise correlation volume) Bass kernel for Trainium2.

Problem: left/right features [2, 320, 96, 312] fp32, GROUP=40, cpg=8,
max_disp=48.  Output cost volume [2, 40, 48, 96, 312]:
    cost[b,g,d,h,w] = mean_c( l[b,g,c,h,w] * r[b,g,c,h,w-d] ),  0 for w<d.

Strategy (8 NeuronCores):
  - Shard the 80 (b,g) pairs across cores, 10 per core.  Each pair is fully
    independent (no collectives).
  - TensorE does all multiply-accumulate work as block-diagonal matmuls:
    for each (bg, h-group of 16), SBUF holds l as [128 = 16h x 8c, W] and a
    block-diagonal stationary image rs [128, 10*128] where the
    (unit, w'-block blk, h-quad q) stationary is
        rs[32q + 8hi + c, 128 blk + 32 hi + ww] = r[h, c, 32 blk + ww] / 0,
    h = 16 hg + 4 q + hi.  matmul (K=32 rows at strip 32q, M=128, N=79):
        out[(hi,ww), n] = sum_c r[h,c,w'0+ww] * l[h,c,w'0+n]
                        = cost[d=n-ww, h, w=w'0+n]  for 0 <= n-ww < 48.
    The 4 quads run on distinct PE row-strips and distinct PSUM banks,
    so they execute concurrently on the 32x32 sub-array grid.
  - HBM holds only the COMPACT r [128, GU, 320] (4x fewer bytes than the
    block-diagonal image).  The otherwise-idle GPSIMD engine expands it on
    device as a uint32 bitwise-AND against a per-partition hc-selection
    mask (2 bf16 values per op), one instruction per (group, unit).
  - Band compaction via slot-major ev layout: stationary M-cols are
    permuted (in the on-device build) so PSUM partition p = 64k + 16hi + i
    holds ww = 16k + i, making the valid 48-wide band of partitions [0,64)
    sit inside moving cols [0,64) and of [64,128) inside [16,80).
    ev stores the matmul col index ("slot") OUTER-most per partition,
    so each half's 64-slot window is one contiguous run and the
    out-DMA extracts a 64/48 = 1.33x band with ~5KB packets.  (Narrower
    windows would need more dma_starts; at ~1us of engine issue time per
    dma_start, instruction count dominates any byte savings.)
  - PSUM is evacuated by vector+scalar with slot ranges split 42/38 to
    balance their measured copy rates (gpsimd cannot read PSUM).
  - The host does the final (free) rearrangement: band extraction
    (d = n - ww), zero triangle for w < d, and the layout transpose.
"""

import os

import numpy as np

# --- geometry (hardcoded for this problem) ---
B, G, CPG, H, W = 2, 40, 8, 96, 312
D = 48                      # max_disp
N_CORES = 8
PAIRS = B * G               # 80 (b,g) pairs
BG_PER_CORE = PAIRS // N_CORES  # 10
HGROUPS = H // 16           # 6 groups of 16 h's
NBLK = 10                   # w'-blocks of 32 (covers w' in [0, 320))
MBLK = 32                   # w' per block
NW = MBLK + D - 1           # 79 moving columns per matmul
WL = 368                    # padded l width (312 + 56; max needed w = 366)
WR = 320                    # padded r width (312 + 8)
UNITS = BG_PER_CORE * HGROUPS   # 60 (bg, hgroup) units per core
RSW = NBLK * 128            # 1280 stationary-image cols per unit
GU = 6                      # units per DMA group (batch DMAs for efficiency)
NGROUPS = UNITS // GU       # 10

_NC_CACHE = {}


def _build_nc(dt_in_name="bfloat16", dt_out_name="bfloat16"):
    from concourse import bacc, mybir, tile
    import concourse.bass as bass  # noqa: F401

    dt_in = getattr(mybir.dt, dt_in_name)
    dt_out = getattr(mybir.dt, dt_out_name)
    f32 = mybir.dt.float32
    u32 = mybir.dt.uint32

    nc = bacc.Bacc("TRN2", target_bir_lowering=False, debug=False)
    l_dram = nc.dram_tensor(
        "l", [NGROUPS, 128, GU, WL], dt_in, kind="ExternalInput")
    r_dram = nc.dram_tensor(
        "rc", [NGROUPS, 128, GU, WR], dt_in, kind="ExternalInput")
    m_dram = nc.dram_tensor("mk", [128, 4], dt_in, kind="ExternalInput")
    NWM = 80                    # matmul cols (covers window [16,80))
    o_dram = nc.dram_tensor(
        "o", [NGROUPS, 2, 64, GU, 64, 4, NBLK], dt_out,
        kind="ExternalOutput")

    HB = NBLK // 2              # 5 blks per psum tile (one bank per quad)
    SEG = 96                    # col stride of a blk inside its bank
    GH = GU // 2                # out-DMA half-group granularity
    # 2-way evac slot split (vector / scalar; gpsimd cannot read PSUM),
    # tuned for the engines' measured copy rates (~1.25 / 1.41 ns/el).
    SPLITS = (0, 42, NWM)

    with tile.TileContext(nc) as tc:
        with (
            tc.tile_pool(name="mp", bufs=1) as mp,
            tc.tile_pool(name="lp", bufs=2) as lp,
            tc.tile_pool(name="cp", bufs=3) as cp,
            tc.tile_pool(name="rp", bufs=3) as rp,
            tc.tile_pool(name="evp", bufs=3) as evp,
            tc.tile_pool(name="psp", bufs=2, space="PSUM") as psp,
        ):
            mask = mp.tile([128, 4], dt_in)
            nc.sync.dma_start(mask[:], m_dram[:])
            for g in range(NGROUPS):
                lt = lp.tile([128, GU, WL], dt_in)
                ct = cp.tile([128, GU, WR], dt_in)
                rt = rp.tile([128, GU, RSW], dt_in)
                # split loads in halves: the first half-group's build +
                # compute can start while the second half streams in
                # all input-load issues on the (mostly idle) sync engine:
                # the scalar engine is busy with evac copies and would
                # delay next-group loads behind them in program order
                for u0 in (0, GH):
                    nc.sync.dma_start(
                        ct[:, u0:u0 + GH, :], r_dram[g, :, u0:u0 + GH, :])
                    nc.sync.dma_start(
                        lt[:, u0:u0 + GH, :], l_dram[g, :, u0:u0 + GH, :])
                # gpsimd expands compact r to the (slot-major permuted)
                # block-diagonal image, one broadcast-multiply per unit:
                #   rs[p, u, blk, k, hc, i] = rc[p, u, blk, k, i]
                #                             * mask[p, hc]
                # (Pool cannot do uint32 bitwise ops, so bf16 multiply.)
                for u in range(GU):
                    # (blk, k) merged into one dim: engine APs allow at
                    # most 3 free dims (TENSOR3D)
                    src = ct[:, u].rearrange(
                        "p (bk i) -> p bk i", bk=2 * NBLK)
                    src_b = src.unsqueeze(2).broadcast_to(
                        [128, 2 * NBLK, 4, 16])
                    msk_b = mask.unsqueeze(1).unsqueeze(3).broadcast_to(
                        [128, 2 * NBLK, 4, 16])
                    dst = rt[:, u].rearrange(
                        "p (bk hc i) -> p bk hc i", bk=2 * NBLK, hc=4)
                    nc.gpsimd.tensor_tensor(
                        out=dst, in0=src_b, in1=msk_b,
                        op=mybir.AluOpType.mult)
                ev = evp.tile([128, GU, NWM, 4, NBLK], dt_out)
                for ui in range(GU):
                    for h in range(2):
                        # 5 blks share one psum tile: quad q in bank q,
                        # blk s at col 96*s
                        ps = psp.tile([128, 4, 512], f32)
                        for s in range(HB):
                            blk = h * HB + s
                            for q in range(4):
                                nc.tensor.matmul(
                                    out=ps[:, q, SEG * s:SEG * s + NWM],
                                    lhsT=rt[32 * q:32 * q + 32, ui,
                                            128 * blk:128 * blk + 128],
                                    rhs=lt[32 * q:32 * q + 32, ui,
                                           MBLK * blk:MBLK * blk + NWM],
                                    start=True,
                                    stop=True,
                                    tile_position=(32 * q, 0),
                                )
                        # [p, q, s, n] -> [p, n, q, s] slot-major store,
                        # 2-way slot split across vector/scalar
                        src = ps[:, :, 0:HB * SEG].rearrange(
                            "p q (s n) -> p n q s", n=SEG)
                        dst = ev[:, ui, :, :, h * HB:(h + 1) * HB]
                        for ei, eng_copy in enumerate(
                                (nc.vector.tensor_copy, nc.scalar.copy)):
                            s0, s1 = SPLITS[ei], SPLITS[ei + 1]
                            eng_copy(out=dst[:, s0:s1],
                                     in_=src[:, s0:s1])
                    # out-DMA per (half-group, partition-half); each reads
                    # its half's contiguous 64-slot band window.  k=0
                    # targets even SDMA engines (p<64) and k=1 odd
                    # (p>=64): issue them on SEPARATE HWDGE rings
                    # (ACT / sync) so both engine-parity sets drain
                    # concurrently from independent descriptor streams
                    if ui == GH - 1 or ui == GU - 1:
                        u0 = ui + 1 - GH
                        for k in range(2):
                            eng = nc.scalar if k == 0 else nc.sync
                            eng.dma_start(
                                o_dram[g, k, :, u0:u0 + GH],
                                ev[64 * k:64 * k + 64, u0:u0 + GH,
                                   16 * k:16 * k + 64, :, :])
    nc.compile()
    return nc


def _get_nc(key=("bfloat16", "bfloat16")):
    if key not in _NC_CACHE:
        _NC_CACHE[key] = _build_nc(*key)
    return _NC_CACHE[key]


def _pack_inputs(left, right, dt_np):
    """-> per-core in_maps; l pre-scaled by 1/cpg (padded to 368 cols),
    r compact (padded to 320), both [NGROUPS, 128 = 16h x 8c, GU, W*],
    plus the uint32 block-diagonal selection mask [128, 4]."""
    # [B, C, H, W] -> [B, G, cpg, H, W] -> [pair, H, cpg, W]
    l5 = left.reshape(B, G, CPG, H, W).transpose(0, 1, 3, 2, 4).reshape(
        PAIRS, H, CPG, W)
    r5 = right.reshape(B, G, CPG, H, W).transpose(0, 1, 3, 2, 4).reshape(
        PAIRS, H, CPG, W)
    lp = np.zeros((PAIRS, H, CPG, WL), dtype=np.float32)
    lp[..., :W] = l5 * (1.0 / CPG)
    lp = lp.astype(dt_np)
    # l: [pair, H=6*16, cpg, WL] -> per core [NGROUPS, 128, GU, WL]
    lp = lp.reshape(N_CORES, NGROUPS, GU, 128, WL).transpose(0, 1, 3, 2, 4)

    rp = np.zeros((PAIRS, H, CPG, WR), dtype=dt_np)
    rp[..., :W] = r5
    rp = rp.reshape(N_CORES, NGROUPS, GU, 128, WR).transpose(0, 1, 3, 2, 4)

    mk = np.zeros((128, 4), dtype=dt_np)
    for q in range(4):
        for hi in range(4):
            mk[32 * q + 8 * hi:32 * q + 8 * hi + 8, hi] = 1
    return [
        {"l": np.ascontiguousarray(lp[k]),
         "rc": np.ascontiguousarray(rp[k]),
         "mk": mk}
        for k in range(N_CORES)
    ]


def _unpack_outputs(outs):
    """outs: 8 arrays [NGROUPS, 2, 64, GU, 64, 4, NBLK] -> [B,G,D,H,W]."""
    # dims: [g, k, (4hi,16i), ui, 64j, 4q, 10blk]; partition held
    # ww = 16k + i, stored slot j maps to matmul col n = 16k + j, so
    # d = n - ww = j - i (valid j in [i, i+48))
    O = np.stack([
        np.asarray(o, dtype=np.float32).transpose(0, 3, 1, 2, 4, 5, 6)
        for o in outs])
    # [8core, NG, GU, 2k, 4hi, 16i, 64j, 4q, 10blk] -> unit-major
    O = O.reshape(PAIRS, HGROUPS, 2, 4, 16, 64, 4, NBLK)
    WPAD = 368
    final = np.zeros((PAIRS, D, H, WPAD), dtype=np.float32)
    s0, sd, sh, sw = (np.array(final.strides) // final.itemsize)
    st = np.lib.stride_tricks.as_strided
    it = final.itemsize
    for q in range(4):
        for hi in range(4):
            for k in range(2):
                h0 = 4 * q + hi
                A = O[:, :, k, hi, :, :, q, :]  # [80, 6, 16i, 64j, 10blk]
                a = np.array(A.strides) // it
                # V[pair, hg, i, blk, d] = A[pair, hg, i, i+d, blk]
                V = st(A, shape=(PAIRS, HGROUPS, 16, NBLK, D),
                       strides=tuple(np.array([a[0], a[1], a[2] + a[3],
                                               a[4], a[3]]) * it))
                # dest: final[pair, d, 16*hg + h0, 32*blk + 16*k + i + d]
                Dv = st(final[:, :, h0:, 16 * k:],
                        shape=(PAIRS, HGROUPS, 16, NBLK, D),
                        strides=tuple(np.array([s0, 16 * sh, sw, MBLK * sw,
                                                sd + sw]) * it))
                Dv[...] = V
    return final[:, :, :, :W].reshape(B, G, D, H, W)


def _install_profile_hook():
    """Make trace=True work when the image's antenv lacks axon_hooks."""
    import sys
    import types
    try:
        from antenv.axon_hooks import get_axon_ntff_profile_hook  # noqa: F401
        return
    except ImportError:
        pass
    if "/root/.axon_site" not in sys.path:
        sys.path.insert(0, "/root/.axon_site")
    from trn_agent_boot.trn_boot import _ntff_profile_via_ctypes
    hook = _ntff_profile_via_ctypes("/opt/axon/libaxon_pjrt.so")
    import antenv
    mod = types.ModuleType("antenv.axon_hooks")
    state = {"hook": hook}
    mod.get_axon_ntff_profile_hook = lambda: state["hook"]
    mod.set_axon_ntff_profile_hook = lambda h: state.update(hook=h)
    sys.modules["antenv.axon_hooks"] = mod
    antenv.axon_hooks = mod


def kernel(left_feature, right_feature, max_disp):
    import sys
    if "/opt/trn_rl_repo" not in sys.path:
        sys.path.insert(0, "/opt/trn_rl_repo")
    from concourse import bass_utils
    from concourse.bass_utils import run_bass_kernel_spmd

    left = np.asarray(left_feature, dtype=np.float32)
    right = np.asarray(right_feature, dtype=np.float32)
    assert int(max_disp) == D
    assert left.shape == (B, G * CPG, H, W)

    dt_in_name = os.environ.get("GWC_DT_IN", "bfloat16")
    dt_out_name = os.environ.get("GWC_DT_OUT", "bfloat16")
    if dt_in_name == "bfloat16":
        import ml_dtypes
        dt_np = ml_dtypes.bfloat16
    else:
        dt_np = np.float32
    nc = _get_nc((dt_in_name, dt_out_name))
    in_maps = _pack_inputs(left, right, dt_np)

    trace = bool(os.environ.get("GWC_PROFILE"))
    if trace:
        _install_profile_hook()
        bass_utils.upload_artifacts = lambda tmpdir: str(tmpdir)  # no bucket
    res = run_bass_kernel_spmd(
        nc, in_maps, core_ids=list(range(N_CORES)), trace=trace
    )
    if trace:
        kernel._last_profile = res
        print(f"[kernel] exec_time_ns={res.exec_time_ns} "
              f"mean={res.mean_exec_time_ns}", flush=True)
    outs = [res.results[k]["o"] for k in range(N_CORES)]
    return _unpack_outputs(outs)
